# revision 34
# baseline (speedup 1.0000x reference)
# Trainium2 Bass kernel for nn_MultiHeadAttention (B=2, S=2048, D=1024, H=16).
#
# Sharding: batch x head-quad over 8 cores. Core c handles batch c//4 and
# heads [4*(c%4), 4*(c%4)+4) (EL=256 local e-dims). Each core streams only
# its batch's q/k/v (12MB vs 24MB for pure head sharding), computes partial
# outputs over its 256 e-dims, and the host sums the 4 partials per batch.
#
# Device pipeline (per core, tokens T=2048, d_k=64):
#   QT/KT = (128, 2 groups, T) transposed activations via weight-stationary
#   matmuls; bq is fused into the QT PSUM->SBUF copy. bk is dropped entirely
#   (a per-q-row constant in the scores - softmax invariant) and bv is folded
#   into bo on the host (attn rows sum to 1), so K/V projections are pure.
#   V is projected directly into natural (token, e) layout by using the input
#   chunk as the stationary operand - no transposes; V is stored per k-tile
#   as 4x[64 dims | ones-col] so the AV matmul also produces the softmax
#   denominator Z for free.
#   Scores are computed transposed, scT = (k, q); exp runs PSUM->SBUF on ACT
#   producing P^T tiles. The AV matmul uses P^T slices as the *stationary*
#   operand and V-natural as the moving operand, so it is charged only
#   65 rows per (kt, qtile) instead of 512: out lands in natural (q, v)
#   layout. Normalization is a per-partition scalar multiply (no broadcasts),
#   then the XBAR DMA transpose restores (e, t) layout for the output
#   projection, which streams partial (t, D) results to HBM in fp16.
#   Causal masking: -1e30 bias added in-PSUM via an identity-weight matmul
#   over only the masked column window of each diagonal block.
# Scheduling: PE is strictly in-order, so the emitter software-pipelines
#   attention pairs (QK of pair i ahead of AV of pair i-1, hiding the ACT
#   exp latency) and spreads projection/output-projection matmuls as filler
#   units between pairs so PE keeps running while ACT works through exps.
# Dtypes: streams/weights/attention fp16, mask bias bf16, PSUM f32, exp f32.

import numpy as np

B, S, D, H = 2, 2048, 1024, 16
DK = D // H            # 64
NC = 8                 # cores
CPB = 4                # cores per batch
NH = H // CPB          # 4 heads per core
EL = NH * DK           # 256 local e-dims
T = S                  # tokens per core (one batch)
NCH = 4                # projection chunks of 512 tokens
CH = T // NCH          # 512
NDT = D // 128         # 8 contraction tiles
NKT = T // 128         # 16 k-tiles
NQB = T // CH          # 4 q-blocks of 512
NEG = np.float32(-1.0e30)

SKIP, PLAIN = -1, -2   # block classes (>=0 means bias tile index)


def _classify_mask(mask):
    """Per (kt, qj) block classification of the (S_q, S_k) mask.

    Returns (cls[kt][qj], rng live col range, mwin masked-col window,
    bias_blocks (U,128,512) in transposed (k,q) orientation)."""
    m = np.asarray(mask).reshape(S, S)          # [q, k]; 0 = masked
    biasT = np.where(m == 0, NEG, np.float32(0)).T  # [k, q]
    biasT = np.ascontiguousarray(biasT)
    cls = [[PLAIN] * NQB for _ in range(NKT)]
    rng = [[(0, CH)] * NQB for _ in range(NKT)]
    mwin = [[(0, 0)] * NQB for _ in range(NKT)]
    uniq = {}
    blocks = []
    for kt in range(NKT):
        for qj in range(NQB):
            blk = biasT[kt * 128:(kt + 1) * 128, qj * CH:(qj + 1) * CH]
            if not (blk != 0).any():
                cls[kt][qj] = PLAIN
            elif (blk != 0).all():
                cls[kt][qj] = SKIP
            else:
                key = blk.tobytes()
                if key not in uniq:
                    uniq[key] = len(blocks)
                    blocks.append(np.ascontiguousarray(blk))
                cls[kt][qj] = uniq[key]
                live = ~(blk != 0).all(axis=0)   # cols with any unmasked
                nz = np.nonzero(live)[0]
                c0, c1 = int(nz[0]), int(nz[-1]) + 1
                rng[kt][qj] = (c0, c1)
                # cols (within live range) containing any masked entry
                haz = (blk[:, c0:c1] != 0).any(axis=0)
                hz = np.nonzero(haz)[0]
                if len(hz):
                    mwin[kt][qj] = (c0 + int(hz[0]), c0 + int(hz[-1]) + 1)
                else:
                    cls[kt][qj] = PLAIN
    if blocks:
        mb = np.stack(blocks)
    else:
        mb = np.zeros((0, 128, CH), np.float32)
    return cls, rng, mwin, mb


def _build_program(cls, rng, mwin, n_bias):
    import concourse.bacc as bacc
    import concourse.mybir as mybir
    from concourse.tile import TileContext

    f32 = mybir.dt.float32
    f16 = mybir.dt.float16
    bf16 = mybir.dt.bfloat16
    Exp = mybir.ActivationFunctionType.Exp

    nc = bacc.Bacc("TRN2", target_bir_lowering=False, debug=False,
                   num_devices=NC)

    qT = nc.dram_tensor("qT", [D, T], f16, kind="ExternalInput")
    kT = nc.dram_tensor("kT", [D, T], f16, kind="ExternalInput")
    vT = nc.dram_tensor("vT", [D, T], f16, kind="ExternalInput")
    w3d = nc.dram_tensor("w3", [128, 3 * NDT * EL], f16,
                         kind="ExternalInput")
    woTd = nc.dram_tensor("woT", [128, 2 * D], f16, kind="ExternalInput")
    bqd = nc.dram_tensor("bq2", [128, 2], f32, kind="ExternalInput")
    # cstb = [ident | resident maskbias blocks] in bf16; large block sets
    # stream from mbias instead
    resident = n_bias <= 12
    n_res = n_bias if resident else 0
    cstbd = nc.dram_tensor("cstb", [128, 128 + n_res * CH], bf16,
                           kind="ExternalInput")
    if not resident:
        mbiasd = nc.dram_tensor("mbias", [n_bias, 128, CH], bf16,
                                kind="ExternalInput")
    out = nc.dram_tensor("out", [T, D], f16, kind="ExternalOutput")

    # transposed-input views: [p, t, c] with t the 128-row block
    qT_r = qT.ap().rearrange("(t p) c -> p t c", p=128)
    kT_r = kT.ap().rearrange("(t p) c -> p t c", p=128)
    vT_r = vT.ap().rearrange("(t p) c -> p t c", p=128)
    w3_r = w3d.ap().rearrange("p (j t e) -> p j t e", j=3, t=NDT)

    with TileContext(nc) as tc:
        with (
            tc.tile_pool(name="const", bufs=1) as constp,
            tc.tile_pool(name="per", bufs=1) as perp,
            tc.tile_pool(name="stage", bufs=3) as stagep,
            tc.tile_pool(name="pt", bufs=4) as ptp,
            tc.tile_pool(name="otn", bufs=2) as otnp,
            tc.tile_pool(name="zr", bufs=4) as zrp,
            tc.tile_pool(name="osr", bufs=6) as osrp,
            tc.tile_pool(name="psS", bufs=2, space="PSUM") as psS,
            tc.tile_pool(name="psAV", bufs=2, space="PSUM") as psAV,
            tc.tile_pool(name="psP", bufs=2, space="PSUM") as psP,
        ):
            # ---- constants (w3 split per projection so matmuls start asap)
            w3 = constp.tile([128, 3, NDT, EL], f16, tag="w3")
            nc.sync.dma_start(out=w3[:, 0, 0:2], in_=w3_r[:, 0, 0:2])
            bq2 = constp.tile([128, 2], f32, tag="bq2")
            cstb = constp.tile([128, 128 + n_res * CH], bf16, tag="cstb")
            identb = cstb[:, 0:128]
            mb_sb = [cstb[:, 128 + u * CH:128 + (u + 1) * CH]
                     for u in range(n_res)]
            woT_sb = constp.tile([128, 2, D], f16, tag="wo")

            # ---- persistent activations ----
            QT_sb = perp.tile([128, 2, T], f16, tag="QT")
            KT_sb = perp.tile([128, 2, T], f16, tag="KT")
            # V natural per k-tile: 4 heads x [64 dims | ones] (65 cols)
            V_sb = perp.tile([128, NKT, NH, 65], f16, tag="V")
            OT_sb = perp.tile([128, 2, T], f16, tag="OT")

            def emit_rest_consts():
                nc.sync.dma_start(out=cstb[:], in_=cstbd.ap()[:])
                nc.sync.dma_start(out=woT_sb[:],
                                  in_=woTd.ap().rearrange(
                                      "p (g u) -> p g u", g=2))
                nc.vector.memset(V_sb[:, :, :, 64:65], 1.0)

            # ---- projection chunk: DMA now, matmuls as filler units ----
            def chunk_units(c, first=False, split_v=False):
                """Emit the chunk's stage DMA; return filler closures, each
                ~0.85us of PE work (proj matmul half-groups + copies).
                With split_v, returns (qk_units, v_units) so the V
                projection can be deferred into the next attention block."""
                sts = []
                for j, src_r in enumerate((qT_r, kT_r, vT_r)):
                    st = stagep.tile([128, NDT, CH], f16, tag="stage",
                                     name=f"st{c}_{j}")
                    if first and j == 0:
                        # interleave weight/stage splits so the first
                        # matmul can start after ~0.7MB of DMA
                        nc.sync.dma_start(
                            out=st[:, 0:2, :],
                            in_=src_r[:, 0:2, c * CH:(c + 1) * CH])
                        nc.sync.dma_start(out=w3[:, 0, 2:NDT],
                                          in_=w3_r[:, 0, 2:NDT])
                        nc.sync.dma_start(
                            out=st[:, 2:NDT, :],
                            in_=src_r[:, 2:NDT, c * CH:(c + 1) * CH])
                        nc.sync.dma_start(out=bq2[:], in_=bqd.ap()[:])
                    elif first:
                        nc.sync.dma_start(out=w3[:, j, 0:4],
                                          in_=w3_r[:, j, 0:4])
                        nc.sync.dma_start(
                            out=st[:, 0:4, :],
                            in_=src_r[:, 0:4, c * CH:(c + 1) * CH])
                        nc.sync.dma_start(out=w3[:, j, 4:NDT],
                                          in_=w3_r[:, j, 4:NDT])
                        nc.sync.dma_start(
                            out=st[:, 4:NDT, :],
                            in_=src_r[:, 4:NDT, c * CH:(c + 1) * CH])
                    else:
                        nc.sync.dma_start(
                            out=st[:], in_=src_r[:, :, c * CH:(c + 1) * CH])
                    if first and j == 2:
                        emit_rest_consts()
                    sts.append(st)
                units = []
                for j in range(2):            # Q, K: e on partitions
                    st = sts[j]
                    if first and False:
                        # t-major across both e-groups: each matmul fires
                        # as its stage d-tile lands
                        def mm_tmaj(j=j, c=c, st=st):
                            pss = [psP.tile([128, CH], f32, tag="pp",
                                            name=f"pp0_{j}_{g}")
                                   for g in range(2)]
                            for t in range(NDT):
                                for g in range(2):
                                    nc.tensor.matmul(
                                        pss[g][:],
                                        w3[:, j, t, g * 128:(g + 1) * 128],
                                        st[:, t, :],
                                        start=(t == 0), stop=(t == NDT - 1))
                            for g in range(2):
                                dsl = (QT_sb if j == 0
                                       else KT_sb)[:, g, c * CH:(c + 1) * CH]
                                if j == 0:
                                    nc.vector.tensor_scalar_add(
                                        dsl, pss[g][:], bq2[:, g:g + 1])
                                else:
                                    nc.vector.tensor_copy(dsl, pss[g][:])

                        units.append(mm_tmaj)
                        continue
                    for g in range(2):
                        ps_box = []

                        def mm_a(j=j, g=g, st=st, ps_box=ps_box):
                            ps = psP.tile([128, CH], f32, tag="pp")
                            ps_box.append(ps)
                            for t in range(4):
                                nc.tensor.matmul(
                                    ps[:],
                                    w3[:, j, t, g * 128:(g + 1) * 128],
                                    st[:, t, :], start=(t == 0), stop=False)

                        def mm_b(j=j, g=g, c=c, st=st, ps_box=ps_box):
                            ps = ps_box[0]
                            for t in range(4, NDT):
                                nc.tensor.matmul(
                                    ps[:],
                                    w3[:, j, t, g * 128:(g + 1) * 128],
                                    st[:, t, :],
                                    start=False, stop=(t == NDT - 1))
                            dsl = (QT_sb if j == 0
                                   else KT_sb)[:, g, c * CH:(c + 1) * CH]
                            if j == 0:
                                nc.vector.tensor_scalar_add(
                                    dsl, ps[:], bq2[:, g:g + 1])
                            else:
                                nc.vector.tensor_copy(dsl, ps[:])

                        units += [mm_a, mm_b]
                st = sts[2]
                v_units = []
                for tk in range(4):           # V natural: tokens on parts

                    def mm_v(tk=tk, c=c, st=st):
                        ps = psP.tile([128, CH], f32, tag="pp")
                        pv = ps[:, 0:EL]
                        for t in range(NDT):
                            nc.tensor.matmul(
                                pv, st[:, t, tk * 128:(tk + 1) * 128],
                                w3[:, 2, t, :],
                                start=(t == 0), stop=(t == NDT - 1))
                        dst = V_sb[:, c * 4 + tk, :, 0:64]
                        nc.vector.tensor_copy(
                            dst, ps[:, 0:EL].rearrange(
                                "p (h x) -> p h x", x=64))

                    v_units.append(mm_v)
                if split_v:
                    return units, v_units
                return units + v_units

            # ---- output projection units for one 128-token tile ----
            def oproj_units(qt, tail=False):
                units = []
                osr_box = []

                def mk(uh, qt=qt, tail=tail, osr_box=osr_box):
                    def u():
                        if uh == 0:
                            osr_box.append(
                                osrp.tile([128, D], f16, tag="osr",
                                          name=f"osr{qt}"))
                        osr = osr_box[0]
                        if tail and uh == 1:
                            # scores pool is idle by the tail: borrow it so
                            # the final oprojs run on a 4-deep psum ring
                            pos = psS.tile([128, 1024], f32, tag="score",
                                           name=f"po_t{qt}")
                            po = pos[:, 0:512]
                        else:
                            po = psP.tile([128, 512], f32, tag="pp")
                        for g in range(2):
                            nc.tensor.matmul(
                                po, OT_sb[:, g, qt * 128:qt * 128 + 128],
                                woT_sb[:, g, uh * 512:(uh + 1) * 512],
                                start=(g == 0), stop=(g == 1))
                        dsl = osr[:, uh * 512:(uh + 1) * 512]
                        if tail:
                            # spread the two copy halves over idle engines
                            nc.scalar.copy(dsl[:, 0:256], po[:, 0:256])
                            nc.vector.tensor_copy(dsl[:, 256:512],
                                                  po[:, 256:512])
                        else:
                            nc.vector.tensor_copy(dsl, po)
                        if uh == 1:
                            nc.sync.dma_start(
                                out=out.ap()[qt * 128:(qt + 1) * 128, :],
                                in_=osr[:])
                    return u

                units += [mk(0), mk(1)]
                return units

            # ---- attention: per-(q-block, head) passes so late q-blocks
            # can interleave with earlier ones as their chunks land ----
            qstate = {}

            def q_init(qj):
                acts = [kt for kt in range(NKT) if cls[kt][qj] != SKIP]
                pairs = []
                for p0 in range(0, NKT, 2):
                    pr = [kt for kt in (p0, p0 + 1) if kt in acts]
                    if pr:
                        pairs.append(pr)
                livek = [[kt for kt in acts
                          if rng[kt][qj][0] < (j + 1) * 128
                          and rng[kt][qj][1] > j * 128]
                         for j in range(4)]
                qstate[qj] = {
                    "pairs": pairs, "livek": livek,
                    "otn": otnp.tile([128, 1024], f16, tag="otn",
                                     name=f"otn{qj}"),
                }

            def emit_transp(qj, j):
                otn = qstate[qj]["otn"]
                qlo = qj * CH
                for g in range(2):
                    nc.sync.dma_start_transpose(
                        out=OT_sb[:, g, qlo + j * 128:qlo + (j + 1) * 128],
                        in_=otn[:, g * 512 + j * 128:g * 512 + (j + 1) * 128])

            def emit_transp_g(qj, g):
                # one XBAR dma per (qj, g): out is the 3D [e, qt, t] view;
                # the transpose engine maps in[t, qt*128+e] -> out[e, qt, t]
                otn = qstate[qj]["otn"]
                qlo = qj * CH
                nc.sync.dma_start_transpose(
                    out=OT_sb[:, g, qlo:qlo + CH].rearrange(
                        "p (j t) -> p j t", t=128),
                    in_=otn[:, g * 512:(g + 1) * 512])

            def emit_qk(qj, h, pair):
                qlo = qj * CH
                g, h2 = h // 2, h % 2
                hs = slice(h2 * 64, h2 * 64 + 64)
                p0 = pair[0] & ~1
                sc = psS.tile([128, 1024], f32, tag="score")
                for kt in pair:
                    i = kt - p0
                    c0, c1 = rng[kt][qj]
                    cl = cls[kt][qj]
                    nc.tensor.matmul(
                        sc[:, i * CH + c0:i * CH + c1],
                        KT_sb[hs, g, kt * 128:kt * 128 + 128],
                        QT_sb[hs, g, qlo + c0:qlo + c1],
                        start=True, stop=(cl == PLAIN))
                    if cl >= 0:
                        m0, m1 = mwin[kt][qj]
                        if resident:
                            mb_ap = mb_sb[cl]
                        else:
                            mbt = ptp.tile(
                                [128, CH], bf16, tag="mbs", bufs=3,
                                name=f"mb_{qj}_{h}_{kt}")
                            nc.sync.dma_start(out=mbt[:],
                                              in_=mbiasd.ap()[cl])
                            mb_ap = mbt[:]
                        nc.tensor.matmul(
                            sc[:, i * CH + m0:i * CH + m1],
                            identb, mb_ap[:, m0:m1],
                            start=False, stop=True)
                # exp: split the pair window when the dead gap between
                # the two k-tiles is wide enough to be worth skipping
                pt = ptp.tile([128, 1024], f16, tag="pt")
                w0 = (rng[pair[0]][qj][0], rng[pair[0]][qj][1])
                if len(pair) == 2:
                    w1 = (CH + rng[pair[1]][qj][0],
                          CH + rng[pair[1]][qj][1])
                    if w1[0] - w0[1] >= int(__import__("os").environ.get("KSPLIT", "288")):
                        wins = [w0, w1]
                    else:
                        wins = [(w0[0], w1[1])]
                else:
                    wins = [w0]
                for lo, hi in wins:
                    nc.scalar.activation(pt[:, lo:hi], sc[:, lo:hi],
                                         Exp, scale=0.125)
                return pt

            def emit_av(ps, pair, pt):
                # the four per-qtile chains share one PSUM bank, and a
                # start=True wipes the whole bank's accumulation state:
                # accumulate onto memset zeros instead (skip_group_check
                # silences the simulator's one-group-per-bank rule)
                qj, h = ps["qj"], ps["h"]
                av3 = ps["av3"]
                p0 = pair[0] & ~1
                for kt in pair:
                    i = kt - p0
                    c0, c1 = rng[kt][qj]
                    for j in range(4):
                        a0 = max(c0, j * 128)
                        a1 = min(c1, (j + 1) * 128)
                        if a0 >= a1:
                            continue
                        off = a0 - j * 128
                        nc.tensor.matmul(
                            av3[off:off + (a1 - a0), j, :],
                            pt[:, i * CH + a0:i * CH + a1],
                            V_sb[:, kt, h, :],
                            start=False, stop=False,
                            skip_group_check=True)

            def start_pass(ps):
                qj, h = ps["qj"], ps["h"]
                av = psAV.tile([128, 4 * 65], f32, tag="av",
                               name=f"av{qj}_{h}")
                nc.vector.memset(av[:], 0.0)
                ps["av3"] = av[:].rearrange("p (j x) -> p j x", x=65)
                ps["seen"] = 0
                ps["pulled"] = 0
                ps["npair"] = len(qstate[qj]["pairs"])

            def step_av(ps, pair, pt):
                emit_av(ps, pair, pt)
                ps["seen"] += 1
                fillers = ps["fillers"]
                want = (ps["seen"] * len(fillers)) // max(1, ps["npair"])
                while ps["pulled"] < want:
                    fillers[ps["pulled"]]()
                    ps["pulled"] += 1
                if ps["seen"] == ps["npair"]:
                    finish_pass(ps)

            def finish_pass(ps):
                qj, h = ps["qj"], ps["h"]
                g, h2 = h // 2, h % 2
                otn = qstate[qj]["otn"]
                av3 = ps["av3"]
                # normalize: per-partition 1/Z into OTn natural layout
                zr = zrp.tile([128, 4], f32, tag="zr")
                nc.vector.reciprocal_approx_fast(
                    zr[:].rearrange("p (j x) -> p j x", x=1),
                    av3[:, :, 64:65])
                for j in range(4):
                    col = g * 512 + j * 128 + h2 * 64
                    nc.vector.tensor_scalar_mul(
                        otn[:, col:col + 64], av3[:, j, 0:64],
                        zr[:, j:j + 1])
                if h == NH - 1:
                    for g in range(2):
                        emit_transp_g(qj, g)
                fillers = ps["fillers"]
                while ps["pulled"] < len(fillers):
                    fillers[ps["pulled"]]()
                    ps["pulled"] += 1

            # ---- schedule: attention passes get progressively more
            # ACT(exp)-heavy, so filler matmul supply is shifted late, and
            # q-block 3's passes interleave with q-block 2's (its chunk is
            # projected inline) to keep ACT fed while PE runs fillers.
            # The pair stream is software-pipelined ACROSS pass boundaries:
            # the next pass's first QK is emitted before the previous
            # pass's final AV so PE never head-blocks on the last exp.
            for u in chunk_units(0, first=True):
                u()                       # chunk 0 runs up front
            stream = []
            q_init(0)
            f0 = chunk_units(1)
            for h in range(NH):
                stream.append({"qj": 0, "h": h,
                               "fillers": f0[h * 3:(h + 1) * 3]})
            q_init(1)
            f1 = chunk_units(2)
            for h in range(NH):
                stream.append({"qj": 1, "h": h,
                               "fillers": f1[h * 3:(h + 1) * 3]})
            q_init(2)
            qk3, v3 = chunk_units(3, split_v=True)
            pre = qk3 + v3
            for qt in range(0, 8):
                pre += oproj_units(qt)
            post = []
            for qt in range(8, 12):
                post += oproj_units(qt)
            alloc = [((2, 0), pre[0:4]), ((2, 1), pre[4:8]),
                     (None, 3), ((3, 0), pre[8:14]),
                     ((2, 2), pre[14:18]), ((3, 1), pre[18:24]),
                     ((2, 3), pre[24:26]),
                     ((3, 2), pre[26:28] + post[0:2]),
                     ((3, 3), post[2:8])]
            for key, fl in alloc:
                if key is None:
                    stream.append({"init": fl})
                    continue
                stream.append({"qj": key[0], "h": key[1], "fillers": fl})

            prev = None
            for ps in stream:
                if "init" in ps:
                    q_init(ps["init"])
                    continue
                start_pass(ps)
                for pair in qstate[ps["qj"]]["pairs"]:
                    pt = emit_qk(ps["qj"], ps["h"], pair)
                    if prev is not None:
                        step_av(*prev)
                    prev = (ps, pair, pt)
            if prev is not None:
                step_av(*prev)

            for qt in range(12, 16):
                for u in oproj_units(qt, tail=True):
                    u()

    nc.compile()
    return nc


_CACHE = {}


def kernel(q, k, v, mask, wq, bq, wk, bk, wv, bv, wo, bo):
    from concourse.bass_utils import run_bass_kernel_spmd

    q = np.asarray(q, np.float32)
    k = np.asarray(k, np.float32)
    v = np.asarray(v, np.float32)
    wq = np.asarray(wq, np.float32)
    wk = np.asarray(wk, np.float32)
    wv = np.asarray(wv, np.float32)
    wo = np.asarray(wo, np.float32)
    bq = np.asarray(bq, np.float32)
    bv = np.asarray(bv, np.float32)
    bo = np.asarray(bo, np.float32)

    # bk shifts every score in a q-row equally (softmax invariant): dropped.
    # bv contributes bv @ wo^T to every output row (attn rows sum to 1):
    # folded into the host-side bias.
    bo_eff = bo + wo @ bv

    qTf = [np.ascontiguousarray(q[b].T.astype(np.float16)) for b in range(B)]
    kTf = [np.ascontiguousarray(k[b].T.astype(np.float16)) for b in range(B)]
    vTf = [np.ascontiguousarray(v[b].T.astype(np.float16)) for b in range(B)]

    cls, rng, mwin, mb = _classify_mask(mask)
    key = (tuple(map(tuple, cls)), tuple(map(tuple, rng)),
           tuple(map(tuple, mwin)), len(mb))
    if key not in _CACHE:
        _CACHE[key] = _build_program(cls, rng, mwin, len(mb))
    nc = _CACHE[key]

    import ml_dtypes
    ident_f = np.eye(128, dtype=np.float32)
    resident = len(mb) <= 12
    res_blocks = [mb[u] for u in range(len(mb))] if resident else []
    cstb = np.ascontiguousarray(np.concatenate(
        [ident_f] + res_blocks, axis=1).astype(ml_dtypes.bfloat16))
    mb_bf = (None if resident else
             np.ascontiguousarray(mb.astype(ml_dtypes.bfloat16)))

    def pack_w3(el):
        ws = []
        for w in (wq, wk, wv):
            wt = np.ascontiguousarray(w[el, :].T.astype(np.float16))
            ws.append(wt.reshape(NDT, 128, EL).transpose(1, 0, 2))
        return np.ascontiguousarray(
            np.stack(ws, axis=1).reshape(128, 3 * NDT * EL))

    in_maps = []
    for c in range(NC):
        b, hq = c // CPB, c % CPB
        el = slice(hq * EL, (hq + 1) * EL)
        woT = wo[:, el].T.astype(np.float16)          # [256, 1024]
        m = {
            "qT": qTf[b], "kT": kTf[b], "vT": vTf[b],
            "w3": pack_w3(el),
            "woT": np.ascontiguousarray(
                woT.reshape(2, 128, D).transpose(1, 0, 2).reshape(
                    128, 2 * D)),
            "bq2": np.ascontiguousarray(
                bq[el].reshape(2, 128).T.astype(np.float32)),
            "cstb": cstb,
        }
        if mb_bf is not None:
            m["mbias"] = mb_bf
        in_maps.append(m)

    res = run_bass_kernel_spmd(nc, in_maps, list(range(NC)))
    outs = []
    for b in range(B):
        acc = res.results[b * CPB]["out"].astype(np.float32)
        for i in range(1, CPB):
            acc = acc + res.results[b * CPB + i]["out"]
        outs.append(acc + bo_eff[None, :])
    return np.stack(outs).reshape(B, S, D)
